# revision 5
# baseline (speedup 1.0000x reference)
"""Trainium2 Bass kernel v2 for causal MHA with RoPE (see kernel.py docstring
for the problem statement and sharding).

Differences from the v1 baseline (274us sim / ~283us hw):
  1. Software-pipelined emission: projections(j+1), attention(j) and
     finalize(j-1) interleave in program order so PE fills its exp-wait gaps
     with projection/output-projection matmuls instead of idling (v1 ran the
     phases back-to-back; PE and ACT each idled ~100us).
  2. PV transposed: P-chunks are the stationary operand, [V|ones] the moving
     one (65 cols vs 512 per k-tile) -- PE cost of PV halves.  ctx comes out
     transposed [q, d]; a regular matmul against a constant fp16 identity
     transposes it back (128 cols per 512-q tile).
  3. Normalization via per-partition broadcast: denominators ride in the PV
     ones-column, reciprocal on DVE, applied during the PSUM->SBUF evacuation
     with a stride-0 broadcast AP (v1 burned PE broadcast matmuls + ACT
     copies on this).
  4. ACT runs exp ONLY (one 3D-AP instruction per k-tile; both heads in one
     instr); all copies/evacuations live on DVE.
PSUM budget: scores 2bufs x [128,2,512]f32 (4 banks) + ctxT 2 heads x
[128,4,65]f32 (2 banks) + misc 2bufs x [128,512]f32 (2 banks) = 8 banks.
"""

import sys

if "/opt/trn_rl_repo" not in sys.path:
    sys.path.insert(0, "/opt/trn_rl_repo")

import numpy as np

OT_MODE = "dve"  # out-proj PSUM evacuation: "dve", "pool", or "alt"
EXP_DVE_NTH = 0  # every Nth full k-tile runs exp on DVE (0 = ACT only)
TRI_POOL = False  # causal-mask multiplies on GPSIMD instead of DVE
PV_DEPTH = 2  # software-pipeline depth between scores and PV
CREDIT_PACING = False
XT_SP = False    # issue all xt DMAs from the SP queue (frees Pool)
OT_ACT_NTH = 0   # every Nth out-proj evacuation on ACT instead of DVE
TRANSP_DMA = False  # ctx transpose via DMA xbar instead of PE matmul + DVE evac
UNIT_COST = {"qk": 1900, "v": 600, "recip": 80, "evac": 80, "transp": 250,
             "po": 1000}

# exp(s) ~= (1 + s(c1 + s(c2 + s c3)))^32 for s in [-12, 4]; rel err
# <= 4e-3 at the (negligible-probability) low end, ~1e-4 elsewhere.
EXP_C1 = 0.03125300429919906
EXP_C2 = 0.0004879666088523769
EXP_C3 = 4.647043799499391e-06


def _register_exp_ops():
    """Register the two custom-DVE ops implementing exp via poly+squaring.

    Idempotent; extends concourse.dve_ops' registries the way a new
    in-tree op would (name -> row, spec, self-pinned uops sha).
    """
    import concourse.dve_ops as D
    from concourse.dve_spec import Spec, Src0, C0, C1, C2, One, sq, lower
    from concourse.dve_uop import DveOpSpec
    import numpy as np

    if "EXP32P_ANT" in D._SUB_OPCODE_FOR_NAME:
        by = {op.name: op for op in D.OPS}
        return by["EXP32P_ANT"], by["SQ32_ANT"]

    def ref1(in0, in1, s0, s1, imm2):
        x = in0.astype(np.float32)
        return ((s0 * x + s1) * x + imm2) * x + 1.0

    def ref2(in0, in1, s0, s1, imm2):
        x = in0.astype(np.float32)
        for _ in range(5):
            x = x * x
        return x

    specs = [
        ("EXP32P_ANT", Spec(body=((Src0 * C0 + C1) * Src0 + C2) * Src0 + One,
                            reference=ref1)),
        ("SQ32_ANT", Spec(body=sq(sq(sq(sq(sq(Src0))))), reference=ref2)),
    ]
    ops = []
    for name, spec in specs:
        row = D._CUSTOM_DVE_ROW_BASE + len(D.OPS)
        shas = {}
        for ver in ("v3", "v4"):
            u = lower(spec, ver=ver)
            shas[ver] = DveOpSpec(name=name, opcode=row, uops=u,
                                  rd1_en=False).sha(ver)
        op = D.DveOp(name, spec, subdim=False, uops_sha=shas)
        D.OPS.append(op)
        D.CUSTOM_DVE_SPECS[name] = spec
        D._SUB_OPCODE_FOR_NAME[name] = row
        ops.append(op)
    return ops

S = 4096
D = 1024
NHEADS = 16
DK = 64
NCORES = 8
HPC = NHEADS // NCORES  # 2 heads per core
TT = 512  # q tile width
NT = S // TT  # 8 tiles
KT = 128  # k tile width
THETA = 10000.0


# ----------------------------------------------------------------------------
# Host-side input preparation (identical to v1 + identity matrix)
# ----------------------------------------------------------------------------

def _perm64():
    return np.concatenate([np.arange(0, DK, 2), np.arange(1, DK, 2)])


def _chunk_lhsT(w_rows):
    t = np.ascontiguousarray(w_rows.T)  # [1024 e, 128 d]
    return np.ascontiguousarray(
        np.transpose(t.reshape(8, 128, 128), (1, 0, 2)).reshape(128, 1024)
    )


def _rope_tables():
    inv_freq = THETA ** (-np.arange(0, DK, 2, dtype=np.float64) / DK)  # [32]
    ang = np.arange(S, dtype=np.float64)[None, :] * inv_freq[:, None]  # [32, S]
    cos32 = np.cos(ang)
    sin32 = np.sin(ang)
    cos = np.concatenate([cos32, cos32, cos32, cos32], 0)
    sinp = np.concatenate([sin32, -sin32, sin32, -sin32], 0)
    return cos, sinp


def prepare_core_inputs(x, Wq, Wk, Wv, Wo, core):
    x, Wq, Wk, Wv, Wo = (np.asarray(a) for a in (x, Wq, Wk, Wv, Wo))
    p64 = _perm64()
    h0, h1 = HPC * core, HPC * core + 1
    rows_perm = np.concatenate([h0 * DK + p64, h1 * DK + p64])
    rows_nat = np.arange(HPC * DK * core, HPC * DK * (core + 1))

    wq_eff = Wq[rows_perm, :].astype(np.float64) / np.sqrt(DK)
    wk_eff = Wk[rows_perm, :].astype(np.float64)
    wv_eff = Wv[rows_nat, :].astype(np.float64)
    cos, sinp = _rope_tables()

    tri = np.triu(np.ones((128, 128), dtype=np.float16))  # 1 where k<=q
    xT = np.ascontiguousarray(x[0].T).astype(np.float16)  # [1024, 4096]

    return {
        "xT": xT,
        "wq": _chunk_lhsT(wq_eff).astype(np.float16),
        "wk": _chunk_lhsT(wk_eff).astype(np.float16),
        "wv": _chunk_lhsT(wv_eff).astype(np.float16),
        "wo": np.ascontiguousarray(Wo[:, rows_nat].T).astype(np.float16),
        "cosT": cos.astype(np.float16),
        "sinT": sinp.astype(np.float16),
        "tri": tri,
        "ident": np.eye(128, dtype=np.float16),
    }


# ----------------------------------------------------------------------------
# Numpy emulation of the v2 device dataflow
# ----------------------------------------------------------------------------

def emulate_core(ins):
    h16 = lambda a: a.astype(np.float16).astype(np.float32)
    xT = ins["xT"].astype(np.float32)
    cos, sinp = (ins["cosT"].astype(np.float32), ins["sinT"].astype(np.float32))

    def proj_T(w_lhsT):
        w = w_lhsT.astype(np.float32)
        out = np.zeros((128, S), np.float32)
        for c in range(8):
            out += w[:, 128 * c:128 * (c + 1)].T @ xT[128 * c:128 * (c + 1), :]
        return out

    def rope(ps):
        raw = h16(ps)
        t1 = h16(raw * cos)
        u = h16(raw * sinp)
        t2 = np.empty_like(raw)
        for b in range(4):
            sblk = b ^ 1
            t2[32 * b:32 * b + 32] = u[32 * sblk:32 * sblk + 32]
        return h16(t1 + t2)

    qt = rope(proj_T(ins["wq"]))
    kt = rope(proj_T(ins["wk"]))
    wv = ins["wv"].astype(np.float32)
    v = np.zeros((S, 128), np.float32)
    for c in range(8):
        v += xT[128 * c:128 * (c + 1), :].T @ wv[:, 128 * c:128 * (c + 1)]
    v = h16(v)
    tri = ins["tri"].astype(np.float32)

    outT = np.zeros((D, S), np.float32)
    wo = ins["wo"].astype(np.float32)
    for j in range(NT):
        n_k = 4 * (j + 1)
        # ctxT accumulation per head: [4 chunks][128 q, 65]
        ctxT = np.zeros((2, 4, 128, 65), np.float32)
        for t in range(n_k):
            ks = slice(KT * t, KT * (t + 1))
            dlt = t - 4 * j
            off = 128 * dlt if dlt >= 0 else 0
            qsl = slice(TT * j + off, TT * (j + 1))
            pt = np.zeros((2, 128, TT), np.float32)  # [h, k, q-in-tile]
            for h in range(2):
                st = kt[64 * h:64 * h + 64, ks].T @ qt[64 * h:64 * h + 64, qsl]
                pt[h, :, off:] = h16(np.exp(st))
            if dlt >= 0:
                for h in range(2):
                    seg = slice(off, off + 128)
                    pt[h][:, seg] = h16(pt[h][:, seg] * tri)
            vp = np.concatenate(
                [v[ks, :], np.ones((128, 2), np.float32)], 1)  # [128, 130]
            for c in range(max(dlt, 0), 4):
                for h in range(2):
                    stat = pt[h][:, 128 * c:128 * (c + 1)]  # [128k, 128q]
                    mov = np.concatenate(
                        [vp[:, 64 * h:64 * h + 64], vp[:, 128 + h:129 + h]], 1)
                    ctxT[h, c] += stat.T @ mov
        # finalize
        ctxn = np.zeros((128, TT), np.float32)  # [d(2 heads), q-in-tile]
        for h in range(2):
            for c in range(4):
                rec = 1.0 / ctxT[h, c, :, 64]  # [128 q] fp32
                ctxs = h16(ctxT[h, c, :, 0:64] * rec[:, None])  # [128q, 64]
                ctxn[64 * h:64 * h + 64, 128 * c:128 * (c + 1)] = ctxs.T
        ctxn = h16(ctxn)
        qs = slice(TT * j, TT * (j + 1))
        outT[:, qs] = h16(wo.T @ ctxn)
    return outT


def emulate(x, Wq, Wk, Wv, Wo):
    acc = np.zeros((D, S), dtype=np.float64)
    for core in range(NCORES):
        acc += emulate_core(prepare_core_inputs(x, Wq, Wk, Wv, Wo, core))
    return np.ascontiguousarray(acc.T.astype(np.float32))[None, :, :]


# ----------------------------------------------------------------------------
# Bass kernel
# ----------------------------------------------------------------------------

def build_nc(loop_n=1):
    import contextlib

    import concourse.bacc as bacc
    import concourse.mybir as mybir
    import concourse.tile as tile

    f32 = mybir.dt.float32
    f16 = mybir.dt.float16
    AF = mybir.ActivationFunctionType

    nc = bacc.Bacc("TRN2", target_bir_lowering=False, debug=False,
                   num_devices=NCORES)
    exp_ops = _register_exp_ops() if EXP_DVE_NTH else None

    xT_d = nc.dram_tensor("xT", [D, S], f16, kind="ExternalInput")
    wq_d = nc.dram_tensor("wq", [128, 1024], f16, kind="ExternalInput")
    wk_d = nc.dram_tensor("wk", [128, 1024], f16, kind="ExternalInput")
    wv_d = nc.dram_tensor("wv", [128, 1024], f16, kind="ExternalInput")
    wo_d = nc.dram_tensor("wo", [128, 1024], f16, kind="ExternalInput")
    cos_d = nc.dram_tensor("cosT", [128, S], f16, kind="ExternalInput")
    sin_d = nc.dram_tensor("sinT", [128, S], f16, kind="ExternalInput")
    tri_d = nc.dram_tensor("tri", [128, 128], f16, kind="ExternalInput")
    id_d = nc.dram_tensor("ident", [128, 128], f16, kind="ExternalInput")
    out_d = nc.dram_tensor("outT", [D, S], f16, kind="ExternalOutput")

    with tile.TileContext(nc) as tc:
        with (
            tc.tile_pool(name="const", bufs=1) as const,
            tc.tile_pool(name="xt", bufs=3) as xtp,
            tc.tile_pool(name="work", bufs=3) as work,
            tc.tile_pool(name="pt", bufs=6) as ptp,
            tc.tile_pool(name="stage", bufs=2) as stage,
            tc.tile_pool(name="ot", bufs=3) as otp,
            tc.tile_pool(name="ps_s", bufs=2, space="PSUM") as ps_s,
            tc.tile_pool(name="ps_ctx", bufs=1, space="PSUM") as ps_ctx,
            tc.tile_pool(name="ps_misc", bufs=2, space="PSUM") as ps_misc,
            (tc.For_i(0, loop_n, 1) if loop_n > 1
             else contextlib.nullcontext()),
        ):
            # ---- constants (xt tile 0 is DMA'd first; see prologue) ----
            wq_sb = const.tile([128, 1024], f16, tag="wq")
            wk_sb = const.tile([128, 1024], f16, tag="wk")
            wv_sb = const.tile([128, 1024], f16, tag="wv")
            wo_sb = const.tile([128, 1024], f16, tag="wo")
            cos_sb = const.tile([128, S], f16, tag="cos")
            sin_sb = const.tile([128, S], f16, tag="sin")
            tri_sb = const.tile([128, 128], f16, tag="tri")
            id_sb = const.tile([128, 128], f16, tag="id")


            # persistent activations
            qt_sb = const.tile([128, S], f16, tag="qt")
            kt_sb = const.tile([128, S], f16, tag="kt")
            # V (+ per-head ones cols 128,129): [128 kpart, slot, 130]
            # moving operand for PV of (t, h) = [v_h (64) | ones (1)] built
            # as 2D AP over cols [64h:64h+64] + [128+h].  Simpler: store as
            # [128, slot, 2, 65] with ones at col 64 (one memset).
            v_sb = const.tile([128, S // KT, 2, 65], f16, tag="v")
            nc.vector.memset(v_sb[:, :, :, 64:65], 1.0)

            # finalize staging
            rec_sb = const.tile([128, 2, 4, 1], f32, tag="rec")
            rscr_sb = const.tile([128, 2, 4, 1], f32, tag="rscr")

            AFexp = AF.Exp

            # ---------------- emission helpers ----------------
            def emit_xt_dma(j, queue=None):
                ts = slice(TT * j, TT * (j + 1))
                xt = xtp.tile([128, 8, TT], f16, tag="xt")
                eng = queue if queue is not None else nc.gpsimd
                for c in range(8):
                    eng.dma_start(xt[:, c, :], xT_d[128 * c:128 * (c + 1), ts])
                return xt

            def emit_rope(raw, dst, ts):
                t1 = work.tile([128, TT], f16, tag="t1")
                nc.vector.tensor_mul(t1[:], raw[:], cos_sb[:, ts])
                t2 = work.tile([128, TT], f16, tag="t2")
                for b in range(4):
                    sblk = b ^ 1
                    nc.vector.tensor_mul(
                        t2[32 * b:32 * b + 32],
                        raw[32 * sblk:32 * sblk + 32],
                        sin_sb[32 * sblk:32 * sblk + 32, ts],
                    )
                nc.vector.tensor_add(dst, t1[:], t2[:])

            def emit_proj_qk(j, xt, w_sb, dst):
                ts = slice(TT * j, TT * (j + 1))
                ps = ps_misc.tile([128, TT], f32, tag="m")
                for c in range(8):
                    nc.tensor.matmul(ps[:], w_sb[:, 128 * c:128 * (c + 1)],
                                     xt[:, c, :], start=(c == 0), stop=(c == 7))
                raw = work.tile([128, TT], f16, tag="raw")
                nc.vector.tensor_copy(raw[:], ps[:])
                emit_rope(raw, dst[:, ts], ts)

            def emit_proj_qk_fused(j, xt):
                # Q and K interleaved per x-chunk: each arriving DMA chunk
                # feeds two matmuls (prologue only -- holds both misc slots).
                ts = slice(TT * j, TT * (j + 1))
                psq = ps_misc.tile([128, TT], f32, tag="m")
                psk = ps_misc.tile([128, TT], f32, tag="m")
                for c in range(8):
                    cs = slice(128 * c, 128 * (c + 1))
                    nc.tensor.matmul(psq[:], wq_sb[:, cs], xt[:, c, :],
                                     start=(c == 0), stop=(c == 7))
                    nc.tensor.matmul(psk[:], wk_sb[:, cs], xt[:, c, :],
                                     start=(c == 0), stop=(c == 7))
                for ps, dst in ((psq, qt_sb), (psk, kt_sb)):
                    raw = work.tile([128, TT], f16, tag="raw")
                    nc.vector.tensor_copy(raw[:], ps[:])
                    emit_rope(raw, dst[:, ts], ts)

            def emit_proj_v(j, xt, s4):
                psv = ps_misc.tile([128, TT], f32, tag="m")
                for c in range(8):
                    nc.tensor.matmul(
                        psv[:, 0:128],
                        xt[:, c, 128 * s4:128 * (s4 + 1)],
                        wv_sb[:, 128 * c:128 * (c + 1)],
                        start=(c == 0), stop=(c == 7),
                    )
                slot = 4 * j + s4
                nc.vector.tensor_copy(
                    v_sb[:, slot, :, 0:64],
                    psv[:, 0:128].rearrange("p (h d) -> p h d", h=2),
                )

            # ---------------- attention tile emission ----------------
            def emit_scores(j, t, ctx_state):
                ks = slice(KT * t, KT * (t + 1))
                dlt = t - 4 * j
                off = 128 * dlt if dlt >= 0 else 0
                qsl = slice(TT * j + off, TT * (j + 1))
                pss = ps_s.tile([128, 2, TT], f32, tag="s")
                nc.tensor.matmul(pss[:, 0, off:TT], kt_sb[0:64, ks],
                                 qt_sb[0:64, qsl], start=True, stop=True)
                nc.tensor.matmul(pss[:, 1, off:TT], kt_sb[64:128, ks],
                                 qt_sb[64:128, qsl], start=True, stop=True)
                pt = ptp.tile([128, 2, TT], f16, tag="pt")
                on_dve = (exp_ops is not None and off == 0
                          and t % EXP_DVE_NTH == EXP_DVE_NTH - 1)
                if on_dve:
                    # exp on DVE: poly(s) ~= exp(s/32), then ^32
                    ep = work.tile([128, 2, TT], f32, tag="ep")
                    nc.vector._custom_dve(exp_ops[0], out=ep[:], in0=pss[:],
                                          s0=EXP_C3, s1=EXP_C2, imm2=EXP_C1)
                    nc.vector._custom_dve(exp_ops[1], out=pt[:], in0=ep[:])
                else:
                    nc.scalar.activation(pt[:, :, off:TT], pss[:, :, off:TT],
                                         AFexp)
                ctx_state[t] = (pt, off, dlt)

            def emit_pv(j, t, ctx_state, ctxT):
                pt, off, dlt = ctx_state.pop(t)
                n_k = 4 * (j + 1)
                if dlt >= 0:
                    seg = slice(off, off + 128)
                    eng = nc.gpsimd if TRI_POOL else nc.vector
                    for h in range(2):
                        eng.tensor_mul(pt[:, h, seg], pt[:, h, seg],
                                       tri_sb[:])
                for c in range(max(dlt, 0), 4):
                    for h in range(2):
                        first = (t == 0 and c == 0)
                        last = (t == n_k - 1 and c == 3)
                        nc.tensor.matmul(
                            ctxT[h][:, c, :],
                            pt[:, h, 128 * c:128 * (c + 1)],
                            v_sb[:, t, h, :],
                            start=first, stop=last,
                            skip_group_check=True,
                        )

            # ---------------- finalize units ----------------
            def fin_units(j, ctxT):
                units = []
                ctxs = stage.tile([128, 4, 2, 64], f16, tag="ctxs")
                ctxn = stage.tile([128, 4, 128], f16, tag="ctxn")

                def u_recip(h):
                    def f():
                        nc.vector.reciprocal_approx_accurate(
                            rec_sb[:, h], ctxT[h][:, :, 64:65], rscr_sb[:, h])
                    return f

                def u_evac(h):
                    def f():
                        rec_b = rec_sb[:, h].broadcast_to((128, 4, 64))
                        nc.vector.tensor_mul(
                            ctxs[:, :, h, :], ctxT[h][:, :, 0:64], rec_b)
                    return f

                def u_transp(c):
                    def f():
                        tr = ps_misc.tile([128, TT], f32, tag="m")
                        nc.tensor.matmul(tr[:, 0:128], ctxs[:, c, :, :],
                                         id_sb[:], start=True, stop=True)
                        nc.vector.tensor_copy(ctxn[:, c, :], tr[:, 0:128])
                    return f

                def u_po(m):
                    def f():
                        qs = slice(TT * j, TT * (j + 1))
                        po = ps_misc.tile([128, TT], f32, tag="m")
                        nc.tensor.matmul(po[:], wo_sb[:, 128 * m:128 * (m + 1)],
                                         ctxn[:, :, :], start=True, stop=True)
                        ot = otp.tile([128, TT], f16, tag="ot")
                        eng = {"dve": nc.vector, "pool": nc.gpsimd}.get(
                            OT_MODE, nc.gpsimd if m % 2 else nc.vector)
                        eng.tensor_copy(ot[:], po[:])
                        nc.sync.dma_start(out_d[128 * m:128 * (m + 1), qs], ot[:])
                    return f

                units += [u_recip(0), u_recip(1), u_evac(0), u_evac(1)]
                units += [u_transp(c) for c in range(4)]
                units += [u_po(m) for m in range(8)]
                return units

            def proj_units(j, xt):
                return [
                    lambda: emit_proj_qk(j, xt, wq_sb, qt_sb),
                    lambda: emit_proj_qk(j, xt, wk_sb, kt_sb),
                    lambda: emit_proj_v(j, xt, 0),
                    lambda: emit_proj_v(j, xt, 1),
                    lambda: emit_proj_v(j, xt, 2),
                    lambda: emit_proj_v(j, xt, 3),
                ]

            # ---------------- pipelined driver ----------------
            # Prologue DMA order matters: wq/wk then x tile 0 on the SP
            # queue (first Q matmul needs wq + x chunk 0); bulk consts go on
            # the idle Pool queue in parallel.  Emit only Q/K projections up
            # front -- V(0) units interleave into round 0 (PV of tile t only
            # needs V slot t).
            nc.sync.dma_start(wq_sb[:], wq_d[:])
            nc.sync.dma_start(wk_sb[:], wk_d[:])
            xt0 = emit_xt_dma(0, queue=nc.sync)
            for sb, dr in ((cos_sb, cos_d), (sin_sb, sin_d), (wv_sb, wv_d),
                           (wo_sb, wo_d), (tri_sb, tri_d), (id_sb, id_d)):
                nc.gpsimd.dma_start(sb[:], dr[:])
            emit_proj_qk_fused(0, xt0)
            pending_v0 = proj_units(0, xt0)[2:]
            xt_tiles = {0: xt0}
            if NT > 1:
                xt_tiles[1] = emit_xt_dma(1, queue=nc.sync)

            def rr(a, b):
                out = []
                for i in range(max(len(a), len(b))):
                    if i < len(a):
                        out.append(a[i])
                    if i < len(b):
                        out.append(b[i])
                return out

            prev_fin = None  # finalize units of j-1
            for j in range(NT):
                n_k = 4 * (j + 1)
                ctxT = [ps_ctx.tile([128, 4, 65], f32, tag=f"ctx{h}",
                                    name=f"ctxT{h}")
                        for h in range(2)]
                if j + 2 < NT:
                    xt_tiles[j + 2] = emit_xt_dma(j + 2)
                pu = pending_v0 + (proj_units(j + 1, xt_tiles[j + 1])
                                   if j < NT - 1 else [])
                pending_v0 = []
                units = rr(prev_fin or [], pu)

                ctx_state = {}
                emitted = 0
                # depth-2 software pipeline: scores(t) ... PV(t-2), so PV
                # never waits on exp latency (~1.4us > one tile of PE work)
                for t in range(n_k):
                    emit_scores(j, t, ctx_state)
                    if t > 1:
                        emit_pv(j, t - 2, ctx_state, ctxT)
                    want = (t + 1) * len(units) // n_k
                    while emitted < want:
                        units[emitted]()
                        emitted += 1
                for t in (n_k - 2, n_k - 1):
                    if t in ctx_state:
                        emit_pv(j, t, ctx_state, ctxT)
                while emitted < len(units):
                    units[emitted]()
                    emitted += 1

                prev_fin = fin_units(j, ctxT)

            for u in prev_fin:
                u()

    nc.compile()
    return nc


_NC_CACHE = {}


def kernel(x, Wq, Wk, Wv, Wo):
    from concourse.bass_utils import run_bass_kernel_spmd

    if "nc" not in _NC_CACHE:
        _NC_CACHE["nc"] = build_nc()
    nc = _NC_CACHE["nc"]

    in_maps = [prepare_core_inputs(x, Wq, Wk, Wv, Wo, c) for c in range(NCORES)]
    last_err = None
    for _ in range(3):
        try:
            res = run_bass_kernel_spmd(nc, in_maps, core_ids=list(range(NCORES)))
            break
        except Exception as e:  # noqa: BLE001
            last_err = e
    else:
        raise last_err
    acc = np.zeros((D, S), dtype=np.float32)
    for r in res.results:
        acc += r["outT"].astype(np.float32)
    return np.ascontiguousarray(acc.T)[None, :, :].astype(np.float32)
